# revision 9
# baseline (speedup 1.0000x reference)
"""Trainium2 Bass kernel for multi-head attention (B=2, S=2048, D=1024, H=16, causal, RoPE).

Sharding: tensor-parallel over heads. Each of the 8 cores computes 2 heads
(128 of the 1024 q/k/v dims): QKV projections for its head slice, RoPE,
causal attention, and a partial output projection against its 128-column
slice of o_weight. The host sums the 8 partial outputs (the all-reduce).

Device-side layout choices:
  - Activations live transposed: q/k are [128 (head dims), seq] so the
    scores matmul contracts dh on partitions. RoPE pairs are de-interleaved
    on the host (weight-row permutation) so pair partners sit 32 partitions
    apart; the rotate step is a single 128x128 sign-swap matmul (sperm).
  - Scores run as fp8e4 DoubleRow matmuls at half cost: stationary is
    [64, 2, 128] = (fp8(k), k - fp8(k)) so the k-side quantization error
    cancels; the moving q8 is a stride-0 broadcast over the pair dim.
  - Scores are computed transposed ([sk, sq]) so P = exp(scores) feeds the
    PV matmul directly as the moving operand (bf16); the two heads' score
    matmuls run per-head. V carries a block of 64 ones columns, so the PV
    matmul also emits the softmax denominator replicated across 64
    partitions; normalization is then a plain reciprocal+multiply.
  - V is projected directly into [seq, dh] layout by using the (transposed)
    x tiles as the stationary operand - no on-chip transposes needed.
  - Work is software-pipelined: projection chunks run one chunk ahead of
    attention, output projections trail one chunk behind, both threaded
    through the attention tile loop so TensorE, ScalarE (exp), VectorE and
    GpSimd stay concurrently busy.
  - Partial outputs are written in bf16 (summed in fp32 on the host).
"""

import numpy as np

D_MODEL = 1024
N_HEADS = 16
D_HEAD = 64
THETA = 10000.0
B = 2
S = 2048
N_CORES = 8
BS = B * S  # 4096
NQ = 512    # query chunk width
NK = 128    # key tile width

_RT = {}


def _build():
    if _RT:
        return _RT
    import sys
    try:
        import concourse.bass  # noqa: F401
    except ImportError:
        sys.path.insert(0, "/opt/trn_rl_repo")
    import concourse.mybir as mybir
    import concourse.tile as tile
    from concourse import bacc
    from concourse._compat import axon_active
    from concourse.bass_utils import run_bass_kernel_spmd

    f32 = mybir.dt.float32
    f32r = mybir.dt.float32r
    bf16 = mybir.dt.bfloat16
    fp8 = mybir.dt.float8e4
    EXP = mybir.ActivationFunctionType.Exp
    DR = mybir.MatmulPerfMode.DoubleRow

    nc = bacc.Bacc(
        "TRN2", target_bir_lowering=False, debug=not axon_active(),
        num_devices=N_CORES,
    )

    xT = nc.dram_tensor("xT", [D_MODEL, BS], bf16, kind="ExternalInput").ap()
    wq = nc.dram_tensor("wq", [D_MODEL, 128], bf16, kind="ExternalInput").ap()
    wk = nc.dram_tensor("wk", [D_MODEL, 128], bf16, kind="ExternalInput").ap()
    wv = nc.dram_tensor("wv", [D_MODEL, 128], bf16, kind="ExternalInput").ap()
    wo = nc.dram_tensor("wo", [128, D_MODEL], f32r, kind="ExternalInput").ap()
    trig = nc.dram_tensor("trig", [128, 2, S], f32, kind="ExternalInput").ap()
    sperm = nc.dram_tensor("sperm", [128, 128], f32r, kind="ExternalInput").ap()
    mask128 = nc.dram_tensor("mask128", [128, 128], bf16, kind="ExternalInput").ap()
    y = nc.dram_tensor("y", [BS, D_MODEL], bf16, kind="ExternalOutput").ap()

    with tile.TileContext(nc) as tc:
        with (
            tc.tile_pool(name="singles", bufs=1) as singles,
            tc.tile_pool(name="px", bufs=3) as px,
            tc.tile_pool(name="ptmp", bufs=3) as ptmp,
            tc.tile_pool(name="pp", bufs=4) as pp,
            tc.tile_pool(name="pys", bufs=4) as pys,
            tc.tile_pool(name="pr", bufs=3) as pr,
            tc.tile_pool(name="ps_a", bufs=2, space="PSUM") as ps_a,
            tc.tile_pool(name="ps_s", bufs=2, space="PSUM") as ps_s,
            tc.tile_pool(name="ps_o", bufs=2, space="PSUM") as ps_o,
        ):
            wq_sb = singles.tile([128, 8, 128], bf16, tag="wq")
            wk_sb = singles.tile([128, 8, 128], bf16, tag="wk")
            wv_sb = singles.tile([128, 8, 128], bf16, tag="wv")
            wo_sb = singles.tile([128, D_MODEL], f32r, tag="wo")
            sperm_sb = singles.tile([128, 128], f32r, tag="sperm")
            m128_sb = singles.tile([128, 128], bf16, tag="m128")
            warm_sb = singles.tile([1, 2], f32, tag="warm")
            # q8: rope'd q in fp8, one slot per chunk-within-batch.
            q8_sb = singles.tile([128, 4, NQ], fp8, tag="q8")
            # k8: [head dims, batch parity, (fp8(k), k-fp8(k)), key pos]
            k8_sb = singles.tile([128, 2, 2, S], fp8, tag="k8")
            oT_sb = singles.tile([128, BS], f32r, tag="oT")
            # V tiles: [seq-tile partitions, 32 tiles, 192]: cols 0:64 head A,
            # 64:128 ones, 128:192 head B. Head A lhsT = cols 0:128, head B
            # lhsT = cols 64:192; the ones block replicates the denominator.
            v_sb = singles.tile([128, 32, 192], bf16, tag="v")

            junk_sb = singles.tile([128, 512], f32r, tag="junk")

            # warm the ACT exp table before the first real exp
            nc.vector.memset(warm_sb, 0.0)
            nc.scalar.activation(warm_sb[:, 0:1], warm_sb[:, 1:2], EXP)

            nc.scalar.dma_start(out=wq_sb, in_=wq.rearrange("(a p) m -> p a m", p=128))
            nc.scalar.dma_start(out=sperm_sb, in_=sperm)
            nc.scalar.dma_start(out=wk_sb, in_=wk.rearrange("(a p) m -> p a m", p=128))
            nc.scalar.dma_start(out=wv_sb, in_=wv.rearrange("(a p) m -> p a m", p=128))
            nc.scalar.dma_start(out=m128_sb, in_=mask128)
            nc.vector.memset(v_sb[:, :, 64:128].bitcast(bf16), 1.0)

            # dummy matmuls: keep PE busy during the initial DMAs and ramp
            # the p-state clock before the first real matmul arrives
            nc.gpsimd.memset(junk_sb, 0.0)
            for w in range(6):
                jp = ps_s.tile([128, 2, 512], f32, tag="sps")
                nc.tensor.matmul(jp[:, 0, :], junk_sb[:, 0:128], junk_sb,
                                 start=True, stop=True)

            def late_consts():
                nc.scalar.dma_start(out=wo_sb, in_=wo)

            def proj_pieces(b, c, after_xt=None):
                """QKV projections + rope + V transpose for seq chunk c of
                batch b (512 positions), as a list of closures that can be
                threaded through the attention tile loop."""
                u = 4 * b + c
                s0 = 512 * c
                xt = []
                tmps = {}
                tg = [None]

                def p_load():
                    for h in range(2):
                        xth = px.tile([128, 4, 512], bf16, tag=f"xt{h}")
                        nc.sync.dma_start(
                            out=xth,
                            in_=xT[512 * h:512 * (h + 1), 512 * u:512 * (u + 1)]
                            .rearrange("(a p) n -> p a n", p=128),
                        )
                        xt.append(xth)
                    tg[0] = ptmp.tile([128, 2, 512], f32, tag="tg", name="tg")
                    nc.sync.dma_start(out=tg[0], in_=trig[:, :, s0:s0 + 512])
                    if after_xt is not None:
                        after_xt()

                def xslot(d):
                    return xt[d // 4][:, d % 4, :]

                def p_proj(w_sb, name):
                    def f():
                        ps = ps_a.tile([128, 512], f32, tag="pa")
                        for d in range(8):
                            nc.tensor.matmul(
                                ps, w_sb[:, d, :], xslot(d),
                                start=(d == 0), stop=(d == 7),
                            )
                        tmp = ptmp.tile([128, 512], f32r, tag=f"{name}tmp")
                        nc.scalar.copy(tmp, ps)  # PSUM -> SBUF on ACT
                        tmps[name] = tmp
                    return f

                def p_rope_q():
                    tmp = tmps["q"]
                    sq = ps_a.tile([128, 512], f32, tag="pa")
                    nc.tensor.matmul(sq, sperm_sb, tmp, start=True, stop=True)
                    cs = tg[0][:, 0, :]
                    sn = tg[0][:, 1, :]
                    m1 = ptmp.tile([128, 512], f32, tag="m1")
                    m2 = ptmp.tile([128, 512], f32, tag="m2")
                    nc.gpsimd.tensor_mul(m1, tmp, cs)
                    nc.vector.tensor_mul(m2, sq, sn)
                    nc.gpsimd.tensor_add(q8_sb[:, c, :], m1, m2)

                def p_rope_k():
                    tmp = tmps["k"]
                    sq = ps_a.tile([128, 512], f32, tag="pa")
                    nc.tensor.matmul(sq, sperm_sb, tmp, start=True, stop=True)
                    cs = tg[0][:, 0, :]
                    sn = tg[0][:, 1, :]
                    m1 = ptmp.tile([128, 512], f32, tag="m1")
                    kf = ptmp.tile([128, 512], f32, tag="kf")
                    nc.gpsimd.tensor_mul(m1, tmp, cs)
                    nc.vector.tensor_mul(kf, sq, sn)
                    nc.gpsimd.tensor_add(kf, m1, kf)
                    k8h = k8_sb[:, b % 2, 0, s0:s0 + 512]
                    nc.vector.tensor_copy(k8h, kf)
                    nc.gpsimd.tensor_sub(k8_sb[:, b % 2, 1, s0:s0 + 512], kf, k8h)

                vps = [None]

                def p_vproj(subs):
                    def f():
                        if vps[0] is None:
                            vps[0] = ps_a.tile([128, 4, 128], f32, tag="pa",
                                               name="vps")
                        for sub in subs:
                            for d in range(8):
                                nc.tensor.matmul(
                                    vps[0][:, sub, :],
                                    xslot(d)[:, 128 * sub:128 * (sub + 1)],
                                    wv_sb[:, d, :],
                                    start=(d == 0), stop=(d == 7),
                                )
                    return f

                def p_vstore_a():
                    nc.vector.tensor_copy(v_sb[:, 4 * u:4 * u + 4, 0:64],
                                          vps[0][:, :, 0:64])

                def p_vstore_b():
                    nc.vector.tensor_copy(v_sb[:, 4 * u:4 * u + 4, 128:192],
                                          vps[0][:, :, 64:128])

                return [p_load, p_proj(wq_sb, "q"), p_proj(wk_sb, "k"),
                        p_rope_q, p_vproj((0,)), p_vproj((1,)),
                        p_rope_k, p_vproj((2,)), p_vproj((3,)),
                        p_vstore_a, p_vstore_b]

            def proj_chunk(b, c, after_xt=None):
                for f in proj_pieces(b, c, after_xt):
                    f()

            def oproj_piece(b, c, s4):
                """Output projection for one 128-row seq tile (emitted one
                chunk late, spread across the next chunk's tiles)."""
                row0 = S * b + NQ * c + 128 * s4
                yp = ps_s.tile([128, 2, 512], f32, tag="sps")
                for hn in range(2):
                    nc.tensor.matmul(
                        yp[:, hn, :],
                        oT_sb[:, row0:row0 + 128],
                        wo_sb[:, 512 * hn:512 * (hn + 1)],
                        start=True, stop=True,
                    )
                ys = pys.tile([128, 1024], bf16, tag="ys")
                # split halves across DVE+Pool so the PSUM slot frees fast
                nc.vector.tensor_copy(ys[:, 0:512], yp[:, 0, :])
                nc.gpsimd.tensor_copy(ys[:, 512:1024], yp[:, 1, :])
                nc.sync.dma_start(out=y[row0:row0 + 128, :], in_=ys)

            def attn_chunk(b, c, mids=(), fine_tail=False, group_tail=False):
                """Causal attention for query chunk c of batch b. ``mids`` are
                emitted one per attention tile (pipelined filler work such as
                the previous chunk's output projection). With ``group_tail``
                (final chunk), PV accumulation stops per 128-query column
                group so normalize+oproj+store overlap the remaining tiles."""
                mids = list(mids)
                qsl = slice(S * b + NQ * c, S * b + NQ * (c + 1))
                nt = (NQ // NK) * (c + 1)
                oa = ps_o.tile([128, 512], f32, tag="oacc")
                ob = ps_o.tile([128, 512], f32, tag="oacc")
                pending = []  # (p tile, j, t) awaiting PV matmul
                PV_DEPTH = 2

                def finish_group(g):
                    # group g of this chunk is fully accumulated: normalize,
                    # project, store - all while later tiles still run
                    fs = slice(128 * g, 128 * (g + 1))
                    qs4 = slice(qsl.start + 128 * g, qsl.start + 128 * (g + 1))
                    rra = pr.tile([64, 128], f32, tag="rra")
                    rrb = pr.tile([64, 128], f32, tag="rrb")
                    nc.vector.reciprocal(rra, oa[64:128, fs])
                    nc.vector.tensor_mul(oT_sb[0:64, qs4], oa[0:64, fs], rra)
                    nc.vector.reciprocal(rrb, ob[0:64, fs])
                    nc.vector.tensor_mul(oT_sb[64:128, qs4], ob[64:128, fs], rrb)
                    oproj_piece(b, c, g)

                def pv_flush():
                    p, j, _t = pending.pop(0)
                    w0 = 128 * j
                    if group_tail:
                        for g in range(j, 4):
                            gs = slice(128 * g, 128 * (g + 1))
                            nc.tensor.matmul(
                                oa[:, gs], v_sb[:, 16 * b + _t, 0:128],
                                p[:, 0, gs],
                                start=(_t == 0), stop=(_t == 4 * c + g),
                            )
                            nc.tensor.matmul(
                                ob[:, gs], v_sb[:, 16 * b + _t, 64:192],
                                p[:, 1, gs],
                                start=(_t == 0), stop=(_t == 4 * c + g),
                            )
                        if _t >= 4 * c:
                            finish_group(_t - 4 * c)
                        return
                    nc.tensor.matmul(
                        oa[:, w0:512], v_sb[:, 16 * b + _t, 0:128],
                        p[:, 0, w0:512],
                        start=(_t == 0), stop=(_t == nt - 1),
                    )
                    nc.tensor.matmul(
                        ob[:, w0:512], v_sb[:, 16 * b + _t, 64:192],
                        p[:, 1, w0:512],
                        start=(_t == 0), stop=(_t == nt - 1),
                    )

                for t in range(nt):
                    j = max(0, t - 4 * c)  # within-chunk diagonal offset
                    w0 = 128 * j           # causally-dead query columns
                    sps = ps_s.tile([128, 2, 512], f32, tag="sps")
                    for h in range(2):
                        hs = slice(64 * h, 64 * h + 64)
                        nc.tensor.matmul(
                            sps[:, h, w0:512],
                            k8_sb[hs, b % 2, :, 128 * t:128 * (t + 1)],
                            q8_sb[hs, c, w0:512].unsqueeze(1)
                            .to_broadcast([64, 2, 512 - w0]),
                            start=True, stop=True, perf_mode=DR,
                        )
                    p = pp.tile([128, 2, 512], bf16, tag="p")
                    nc.scalar.activation(
                        p[:, :, w0:512], sps[:, :, w0:512], EXP, scale=0.125,
                    )
                    if t >= 4 * c:  # diagonal tile: mask boundary block
                        pb = p[:, :, w0:w0 + 128]
                        nc.vector.tensor_mul(
                            pb, pb, m128_sb.unsqueeze(1).to_broadcast([128, 2, 128]),
                        )
                    if len(pending) >= PV_DEPTH:
                        pv_flush()
                    pending.append((p, j, t))
                    if mids:
                        mids.pop(0)()
                while pending:
                    pv_flush()
                for m in mids:  # in case nt < len(mids)
                    m()
                if group_tail:
                    return

                # oa rows 64:128 / ob rows 0:64 hold the replicated
                # softmax denominators (from the ones block in V).
                rra = pr.tile([64, 512], f32, tag="rra")
                rrb = pr.tile([64, 512], f32, tag="rrb")
                if fine_tail:
                    # per-seq-tile normalize so the trailing output projection
                    # can start before the whole chunk is normalized
                    for s4 in range(4):
                        fs = slice(128 * s4, 128 * (s4 + 1))
                        qs4 = slice(qsl.start + 128 * s4, qsl.start + 128 * (s4 + 1))
                        nc.vector.reciprocal(rra[:, fs], oa[64:128, fs])
                        nc.vector.tensor_mul(oT_sb[0:64, qs4], oa[0:64, fs],
                                             rra[:, fs])
                        nc.vector.reciprocal(rrb[:, fs], ob[0:64, fs])
                        nc.vector.tensor_mul(oT_sb[64:128, qs4], ob[64:128, fs],
                                             rrb[:, fs])
                else:
                    nc.vector.reciprocal(rra, oa[64:128, :])
                    nc.vector.reciprocal(rrb, ob[0:64, :])
                    nc.vector.tensor_mul(oT_sb[0:64, qsl], oa[0:64, :], rra)
                    nc.vector.tensor_mul(oT_sb[64:128, qsl], ob[64:128, :], rrb)

            # Software pipeline: projections run one chunk ahead of attention;
            # output projections trail their attention chunk by one.
            def oproj_mids(bc):
                if bc is None:
                    return ()
                return [lambda s4=s4: oproj_piece(bc[0], bc[1], s4)
                        for s4 in range(4)]

            prev = None  # (b, c) whose oproj is still owed
            for b in range(B):
                if b == 0:
                    proj_chunk(b, 0, after_xt=late_consts)
                for c in range(4):
                    mids = list(oproj_mids(prev))
                    if c + 1 < 4:
                        pieces = proj_pieces(b, c + 1)
                    elif b + 1 < B:
                        # thread the next batch's first projection through
                        # this batch's last attention chunk
                        pieces = proj_pieces(b + 1, 0)
                    else:
                        pieces = []
                    merged = []
                    while pieces or mids:
                        if pieces:
                            merged.append(pieces.pop(0))
                        if mids:
                            merged.append(mids.pop(0))
                    mids = merged
                    last = b + 1 == B and c == 3
                    attn_chunk(b, c, mids=mids, group_tail=last)
                    prev = (b, c)

    nc.compile()
    _RT.update(
        nc=nc, run_bass_kernel_spmd=run_bass_kernel_spmd, mybir=mybir,
    )
    return _RT


def _host_inputs(q_weight, k_weight, v_weight, o_weight, in_features):
    """Build the per-core input maps (host-side sharding + layout prep)."""
    x = np.ascontiguousarray(np.asarray(in_features, dtype=np.float32))
    qw = np.asarray(q_weight, dtype=np.float32)
    kw = np.asarray(k_weight, dtype=np.float32)
    vw = np.asarray(v_weight, dtype=np.float32)
    ow = np.asarray(o_weight, dtype=np.float32)

    import ml_dtypes
    xT = np.ascontiguousarray(x.reshape(BS, D_MODEL).T).astype(ml_dtypes.bfloat16)

    perm64 = np.concatenate([np.arange(0, 64, 2), np.arange(1, 64, 2)])

    half = D_HEAD // 2
    inv_freq = THETA ** (-(np.arange(half, dtype=np.float64) * 2.0 / D_HEAD))
    pos = np.arange(S, dtype=np.float64)
    ang = pos[None, :] * inv_freq[:, None]        # [32, S]
    angf = np.tile(ang, (4, 1))                   # [128, S], row p -> i = p % 32
    trig = np.ascontiguousarray(np.stack(
        [np.cos(angf), np.sin(angf)], axis=1).astype(np.float32))

    spermT = np.zeros((128, 128), dtype=np.float32)
    for h in range(2):
        for i in range(32):
            spermT[h * 64 + 32 + i, h * 64 + i] = -1.0
            spermT[h * 64 + i, h * 64 + 32 + i] = 1.0

    kq = np.arange(128)
    mask128 = (np.arange(128)[None, :] >= kq[:, None]).astype(ml_dtypes.bfloat16)

    shared = dict(xT=xT, trig=trig, sperm=spermT, mask128=mask128)

    in_maps = []
    for c in range(N_CORES):
        rows = slice(128 * c, 128 * (c + 1))

        def permqk(w):
            wc = w[rows]
            return np.ascontiguousarray(
                np.concatenate([wc[0:64][perm64], wc[64:128][perm64]]).T
            ).astype(ml_dtypes.bfloat16)

        in_maps.append(dict(
            shared,
            wq=permqk(qw),
            wk=permqk(kw),
            wv=np.ascontiguousarray(vw[rows].T).astype(ml_dtypes.bfloat16),
            wo=np.ascontiguousarray(ow[:, rows].T),
        ))
    return in_maps


def kernel(q_weight, k_weight, v_weight, o_weight, in_features):
    rt = _build()
    in_maps = _host_inputs(q_weight, k_weight, v_weight, o_weight, in_features)
    res = rt["run_bass_kernel_spmd"](
        rt["nc"], in_maps, core_ids=list(range(N_CORES)),
    )
    y = np.zeros((BS, D_MODEL), dtype=np.float32)
    for c in range(N_CORES):
        y += np.asarray(res.results[c]["y"], dtype=np.float32)
    return y.reshape(B, S, D_MODEL)


# revision 15
# speedup vs baseline: 1.0264x; 1.0264x over previous
"""Trainium2 Bass kernel for multi-head attention (B=2, S=2048, D=1024, H=16, causal, RoPE).

Sharding: tensor-parallel over heads. Each of the 8 cores computes 2 heads
(128 of the 1024 q/k/v dims): QKV projections for its head slice, RoPE,
causal attention, and a partial output projection against its 128-column
slice of o_weight. The host sums the 8 partial outputs (the all-reduce).

Device-side layout choices:
  - Activations live transposed: q/k are [128 (head dims), seq] so the
    scores matmul contracts dh on partitions. RoPE pairs are de-interleaved
    on the host (weight-row permutation) so pair partners sit 32 partitions
    apart; the rotate step is a single 128x128 sign-swap matmul (sperm).
  - Scores run as fp8e4 DoubleRow matmuls at half cost: stationary is
    [64, 2, 128] = (fp8(k), k - fp8(k)) so the k-side quantization error
    cancels; the moving q8 is a stride-0 broadcast over the pair dim.
  - Scores are computed transposed ([sk, sq]) so P = exp(scores) feeds the
    PV matmul directly as the moving operand (bf16); the two heads' score
    matmuls run per-head. V carries a block of 64 ones columns, so the PV
    matmul also emits the softmax denominator replicated across 64
    partitions; normalization is then a plain reciprocal+multiply.
  - V is projected directly into [seq, dh] layout by using the (transposed)
    x tiles as the stationary operand - no on-chip transposes needed.
  - Work is software-pipelined: projection chunks run one chunk ahead of
    attention, output projections trail one chunk behind, both threaded
    through the attention tile loop so TensorE, ScalarE (exp), VectorE and
    GpSimd stay concurrently busy.
  - Partial outputs are written in bf16 (summed in fp32 on the host).
"""

import numpy as np

D_MODEL = 1024
N_HEADS = 16
D_HEAD = 64
THETA = 10000.0
B = 2
S = 2048
N_CORES = 8
BS = B * S  # 4096
NQ = 512    # query chunk width
NK = 128    # key tile width

_RT = {}


def _build():
    if _RT:
        return _RT
    import sys
    try:
        import concourse.bass  # noqa: F401
    except ImportError:
        sys.path.insert(0, "/opt/trn_rl_repo")
    import concourse.mybir as mybir
    import concourse.tile as tile
    from concourse import bacc
    from concourse._compat import axon_active
    from concourse.bass_utils import run_bass_kernel_spmd

    f32 = mybir.dt.float32
    f32r = mybir.dt.float32r
    bf16 = mybir.dt.bfloat16
    fp8 = mybir.dt.float8e4
    EXP = mybir.ActivationFunctionType.Exp
    DR = mybir.MatmulPerfMode.DoubleRow

    nc = bacc.Bacc(
        "TRN2", target_bir_lowering=False, debug=not axon_active(),
        num_devices=N_CORES,
    )

    xT = nc.dram_tensor("xT", [D_MODEL, BS], bf16, kind="ExternalInput").ap()
    wq = nc.dram_tensor("wq", [D_MODEL, 128], bf16, kind="ExternalInput").ap()
    wk = nc.dram_tensor("wk", [D_MODEL, 128], bf16, kind="ExternalInput").ap()
    wv = nc.dram_tensor("wv", [D_MODEL, 128], bf16, kind="ExternalInput").ap()
    wo = nc.dram_tensor("wo", [128, D_MODEL], f32r, kind="ExternalInput").ap()
    trig = nc.dram_tensor("trig", [128, 2, S], f32, kind="ExternalInput").ap()
    sperm = nc.dram_tensor("sperm", [128, 128], f32r, kind="ExternalInput").ap()
    mask128 = nc.dram_tensor("mask128", [128, 128], bf16, kind="ExternalInput").ap()
    y = nc.dram_tensor("y", [BS, D_MODEL], bf16, kind="ExternalOutput").ap()

    with tile.TileContext(nc) as tc:
        with (
            tc.tile_pool(name="singles", bufs=1) as singles,
            tc.tile_pool(name="px", bufs=3) as px,
            tc.tile_pool(name="ptmp", bufs=3) as ptmp,
            tc.tile_pool(name="pp", bufs=4) as pp,
            tc.tile_pool(name="pys", bufs=4) as pys,
            tc.tile_pool(name="pr", bufs=3) as pr,
            tc.tile_pool(name="ps_a", bufs=2, space="PSUM") as ps_a,
            tc.tile_pool(name="ps_s", bufs=2, space="PSUM") as ps_s,
            tc.tile_pool(name="ps_o", bufs=2, space="PSUM") as ps_o,
        ):
            wq_sb = singles.tile([128, 8, 128], bf16, tag="wq")
            wk_sb = singles.tile([128, 8, 128], bf16, tag="wk")
            wv_sb = singles.tile([128, 8, 128], bf16, tag="wv")
            wo_sb = singles.tile([128, D_MODEL], f32r, tag="wo")
            sperm_sb = singles.tile([128, 128], f32r, tag="sperm")
            m128_sb = singles.tile([128, 128], bf16, tag="m128")
            warm_sb = singles.tile([1, 2], f32, tag="warm")
            # q8: rope'd q in fp8, one slot per chunk-within-batch.
            q8_sb = singles.tile([128, 4, NQ], fp8, tag="q8")
            # k8: [head dims, batch parity, (fp8(k), k-fp8(k)), key pos]
            k8_sb = singles.tile([128, 2, 2, S], fp8, tag="k8")
            oT_sb = singles.tile([128, BS], f32r, tag="oT")
            # V tiles: [seq-tile partitions, 32 tiles, 192]: cols 0:64 head A,
            # 64:128 ones, 128:192 head B. Head A lhsT = cols 0:128, head B
            # lhsT = cols 64:192; the ones block replicates the denominator.
            v_sb = singles.tile([128, 32, 192], bf16, tag="v")

            junk_sb = singles.tile([128, 512], f32r, tag="junk")

            # warm the ACT exp table before the first real exp
            nc.vector.memset(warm_sb, 0.0)
            nc.scalar.activation(warm_sb[:, 0:1], warm_sb[:, 1:2], EXP)

            nc.scalar.dma_start(out=wq_sb, in_=wq.rearrange("(a p) m -> p a m", p=128))
            nc.scalar.dma_start(out=sperm_sb, in_=sperm)
            nc.vector.memset(v_sb[:, :, 64:128].bitcast(bf16), 1.0)

            # dummy matmuls: keep PE busy during the initial DMAs and ramp
            # the p-state clock before the first real matmul arrives
            nc.gpsimd.memset(junk_sb, 0.0)
            for w in range(7):
                jp = ps_s.tile([128, 2, 512], f32, tag="sps")
                nc.tensor.matmul(jp[:, 0, :], junk_sb[:, 0:128], junk_sb,
                                 start=True, stop=True)

            def late_consts():
                # on the sync queue, after chunk 0's x/trig, ordered by need
                nc.sync.dma_start(out=m128_sb, in_=mask128)
                nc.sync.dma_start(out=wk_sb,
                                  in_=wk.rearrange("(a p) m -> p a m", p=128))
                nc.sync.dma_start(out=wv_sb,
                                  in_=wv.rearrange("(a p) m -> p a m", p=128))
                nc.sync.dma_start(out=wo_sb, in_=wo)

            def proj_pieces(b, c, after_xt=None):
                """QKV projections + rope + V transpose for seq chunk c of
                batch b (512 positions), as a list of closures that can be
                threaded through the attention tile loop."""
                u = 4 * b + c
                s0 = 512 * c
                xt = []
                tmps = {}
                tg = [None]

                def p_load():
                    for h in range(2):
                        xth = px.tile([128, 4, 512], bf16, tag=f"xt{h}")
                        nc.sync.dma_start(
                            out=xth,
                            in_=xT[512 * h:512 * (h + 1), 512 * u:512 * (u + 1)]
                            .rearrange("(a p) n -> p a n", p=128),
                        )
                        xt.append(xth)
                    tg[0] = ptmp.tile([128, 2, 512], f32, tag="tg", name="tg")
                    nc.sync.dma_start(out=tg[0], in_=trig[:, :, s0:s0 + 512])
                    if after_xt is not None:
                        after_xt()

                def xslot(d):
                    return xt[d // 4][:, d % 4, :]

                def p_proj(w_sb, name):
                    def f():
                        ps = ps_a.tile([128, 512], f32, tag="pa")
                        for d in range(8):
                            nc.tensor.matmul(
                                ps, w_sb[:, d, :], xslot(d),
                                start=(d == 0), stop=(d == 7),
                            )
                        tmp = ptmp.tile([128, 512], f32r, tag=f"{name}tmp")
                        nc.gpsimd.tensor_copy(tmp, ps)  # PSUM -> SBUF
                        tmps[name] = tmp
                    return f

                def p_rope_q():
                    tmp = tmps["q"]
                    sq = ps_a.tile([128, 512], f32, tag="pa")
                    nc.tensor.matmul(sq, sperm_sb, tmp, start=True, stop=True)
                    cs = tg[0][:, 0, :]
                    sn = tg[0][:, 1, :]
                    m1 = ptmp.tile([128, 512], f32, tag="m1")
                    m2 = ptmp.tile([128, 512], f32, tag="m2")
                    nc.gpsimd.tensor_mul(m1, tmp, cs)
                    nc.vector.tensor_mul(m2, sq, sn)
                    nc.gpsimd.tensor_add(q8_sb[:, c, :], m1, m2)

                def p_rope_k():
                    tmp = tmps["k"]
                    sq = ps_a.tile([128, 512], f32, tag="pa")
                    nc.tensor.matmul(sq, sperm_sb, tmp, start=True, stop=True)
                    cs = tg[0][:, 0, :]
                    sn = tg[0][:, 1, :]
                    m1 = ptmp.tile([128, 512], f32, tag="m1")
                    kf = ptmp.tile([128, 512], f32, tag="kf")
                    nc.gpsimd.tensor_mul(m1, tmp, cs)
                    nc.vector.tensor_mul(kf, sq, sn)
                    nc.gpsimd.tensor_add(kf, m1, kf)
                    k8h = k8_sb[:, b % 2, 0, s0:s0 + 512]
                    nc.vector.tensor_copy(k8h, kf)
                    nc.gpsimd.tensor_sub(k8_sb[:, b % 2, 1, s0:s0 + 512], kf, k8h)

                vps = [None]

                def p_vproj(subs):
                    def f():
                        if vps[0] is None:
                            vps[0] = ps_a.tile([128, 4, 128], f32, tag="pa",
                                               name="vps")
                        for sub in subs:
                            for d in range(8):
                                nc.tensor.matmul(
                                    vps[0][:, sub, :],
                                    xslot(d)[:, 128 * sub:128 * (sub + 1)],
                                    wv_sb[:, d, :],
                                    start=(d == 0), stop=(d == 7),
                                )
                    return f

                def p_vstore_a():
                    nc.vector.tensor_copy(v_sb[:, 4 * u:4 * u + 4, 0:64],
                                          vps[0][:, :, 0:64])

                def p_vstore_b():
                    nc.vector.tensor_copy(v_sb[:, 4 * u:4 * u + 4, 128:192],
                                          vps[0][:, :, 64:128])

                return [p_load, p_proj(wq_sb, "q"), p_proj(wk_sb, "k"),
                        p_rope_q, p_vproj((0,)), p_vproj((1,)),
                        p_rope_k, p_vproj((2,)), p_vproj((3,)),
                        p_vstore_a, p_vstore_b]

            def proj_chunk(b, c, after_xt=None):
                for f in proj_pieces(b, c, after_xt):
                    f()

            def oproj_piece(b, c, s4, late=False):
                """Output projection for one 128-row seq tile (emitted one
                chunk late, spread across the next chunk's tiles)."""
                row0 = S * b + NQ * c + 128 * s4
                yp = ps_s.tile([128, 2, 512], f32, tag="sps")
                for hn in range(2):
                    nc.tensor.matmul(
                        yp[:, hn, :],
                        oT_sb[:, row0:row0 + 128],
                        wo_sb[:, 512 * hn:512 * (hn + 1)],
                        start=True, stop=True,
                    )
                ys = pys.tile([128, 1024], bf16, tag="ys")
                # split halves across two engines so the PSUM slot frees
                # fast; keep ACT free of copies in the exp-bound late chunks
                nc.vector.tensor_copy(ys[:, 0:512], yp[:, 0, :])
                if late:
                    nc.gpsimd.tensor_copy(ys[:, 512:1024], yp[:, 1, :])
                else:
                    nc.scalar.copy(ys[:, 512:1024], yp[:, 1, :])
                nc.sync.dma_start(out=y[row0:row0 + 128, :], in_=ys)

            def attn_chunk(b, c, mids=(), fine_tail=False, group_tail=False):
                """Causal attention for query chunk c of batch b. ``mids`` are
                emitted one per attention tile (pipelined filler work such as
                the previous chunk's output projection). With ``group_tail``
                (final chunk), PV accumulation stops per 128-query column
                group so normalize+oproj+store overlap the remaining tiles."""
                mids = list(mids)
                qsl = slice(S * b + NQ * c, S * b + NQ * (c + 1))
                nt = (NQ // NK) * (c + 1)
                oa = ps_o.tile([128, 512], f32, tag="oacc")
                ob = ps_o.tile([128, 512], f32, tag="oacc")
                pending = []  # (p tile, j, t) awaiting PV matmul
                PV_DEPTH = 2

                def finish_group(g):
                    # group g of this chunk is fully accumulated: normalize,
                    # project, store - all while later tiles still run
                    fs = slice(128 * g, 128 * (g + 1))
                    qs4 = slice(qsl.start + 128 * g, qsl.start + 128 * (g + 1))
                    rra = pr.tile([64, 128], f32, tag="rra")
                    rrb = pr.tile([64, 128], f32, tag="rrb")
                    nc.vector.reciprocal(rra, oa[64:128, fs])
                    nc.vector.tensor_mul(oT_sb[0:64, qs4], oa[0:64, fs], rra)
                    nc.vector.reciprocal(rrb, ob[0:64, fs])
                    nc.vector.tensor_mul(oT_sb[64:128, qs4], ob[64:128, fs], rrb)
                    oproj_piece(b, c, g, late=True)

                def pv_flush():
                    p, j, _t = pending.pop(0)
                    w0 = 128 * j
                    if group_tail:
                        for g in range(j, 4):
                            gs = slice(128 * g, 128 * (g + 1))
                            nc.tensor.matmul(
                                oa[:, gs], v_sb[:, 16 * b + _t, 0:128],
                                p[:, 0, gs],
                                start=(_t == 0), stop=(_t == 4 * c + g),
                            )
                            nc.tensor.matmul(
                                ob[:, gs], v_sb[:, 16 * b + _t, 64:192],
                                p[:, 1, gs],
                                start=(_t == 0), stop=(_t == 4 * c + g),
                            )
                        if _t >= 4 * c:
                            finish_group(_t - 4 * c)
                        return
                    nc.tensor.matmul(
                        oa[:, w0:512], v_sb[:, 16 * b + _t, 0:128],
                        p[:, 0, w0:512],
                        start=(_t == 0), stop=(_t == nt - 1),
                    )
                    nc.tensor.matmul(
                        ob[:, w0:512], v_sb[:, 16 * b + _t, 64:192],
                        p[:, 1, w0:512],
                        start=(_t == 0), stop=(_t == nt - 1),
                    )

                for t in range(nt):
                    j = max(0, t - 4 * c)  # within-chunk diagonal offset
                    w0 = 128 * j           # causally-dead query columns
                    sps = ps_s.tile([128, 2, 512], f32, tag="sps")
                    for h in range(2):
                        hs = slice(64 * h, 64 * h + 64)
                        nc.tensor.matmul(
                            sps[:, h, w0:512],
                            k8_sb[hs, b % 2, :, 128 * t:128 * (t + 1)],
                            q8_sb[hs, c, w0:512].unsqueeze(1)
                            .to_broadcast([64, 2, 512 - w0]),
                            start=True, stop=True, perf_mode=DR,
                        )
                    p = pp.tile([128, 2, 512], bf16, tag="p")
                    nc.scalar.activation(
                        p[:, :, w0:512], sps[:, :, w0:512], EXP, scale=0.125,
                    )
                    if t >= 4 * c:  # diagonal tile: mask boundary block
                        pb = p[:, :, w0:w0 + 128]
                        nc.vector.tensor_mul(
                            pb, pb, m128_sb.unsqueeze(1).to_broadcast([128, 2, 128]),
                        )
                    if len(pending) >= PV_DEPTH:
                        pv_flush()
                    pending.append((p, j, t))
                    if mids:
                        mids.pop(0)()
                while pending:
                    pv_flush()
                for m in mids:  # in case nt < len(mids)
                    m()
                if group_tail:
                    return

                # oa rows 64:128 / ob rows 0:64 hold the replicated
                # softmax denominators (from the ones block in V).
                rra = pr.tile([64, 512], f32, tag="rra")
                rrb = pr.tile([64, 512], f32, tag="rrb")
                if fine_tail:
                    # per-seq-tile normalize so the trailing output projection
                    # can start before the whole chunk is normalized
                    for s4 in range(4):
                        fs = slice(128 * s4, 128 * (s4 + 1))
                        qs4 = slice(qsl.start + 128 * s4, qsl.start + 128 * (s4 + 1))
                        nc.vector.reciprocal(rra[:, fs], oa[64:128, fs])
                        nc.vector.tensor_mul(oT_sb[0:64, qs4], oa[0:64, fs],
                                             rra[:, fs])
                        nc.vector.reciprocal(rrb[:, fs], ob[0:64, fs])
                        nc.vector.tensor_mul(oT_sb[64:128, qs4], ob[64:128, fs],
                                             rrb[:, fs])
                else:
                    nc.vector.reciprocal(rra, oa[64:128, :])
                    nc.vector.reciprocal(rrb, ob[0:64, :])
                    nc.vector.tensor_mul(oT_sb[0:64, qsl], oa[0:64, :], rra)
                    nc.vector.tensor_mul(oT_sb[64:128, qsl], ob[64:128, :], rrb)

            # Software pipeline: projections run one chunk ahead of attention;
            # output projections trail their attention chunk by one.
            def oproj_mids(bc, late=False):
                if bc is None:
                    return ()
                return [lambda s4=s4: oproj_piece(bc[0], bc[1], s4, late=late)
                        for s4 in range(4)]

            prev = None  # (b, c) whose oproj is still owed
            for b in range(B):
                if b == 0:
                    proj_chunk(b, 0, after_xt=late_consts)
                for c in range(4):
                    mids = list(oproj_mids(prev, late=(b == 1 and c >= 2)))
                    if c + 1 < 4:
                        pieces = proj_pieces(b, c + 1)
                    elif b + 1 < B:
                        # thread the next batch's first projection through
                        # this batch's last attention chunk
                        pieces = proj_pieces(b + 1, 0)
                    else:
                        pieces = []
                    merged = []
                    while pieces or mids:
                        if pieces:
                            merged.append(pieces.pop(0))
                        if mids:
                            merged.append(mids.pop(0))
                    mids = merged
                    last = b + 1 == B and c == 3
                    attn_chunk(b, c, mids=mids, group_tail=last)
                    prev = (b, c)

    nc.compile()
    _RT.update(
        nc=nc, run_bass_kernel_spmd=run_bass_kernel_spmd, mybir=mybir,
    )
    return _RT


def _host_inputs(q_weight, k_weight, v_weight, o_weight, in_features):
    """Build the per-core input maps (host-side sharding + layout prep)."""
    x = np.ascontiguousarray(np.asarray(in_features, dtype=np.float32))
    qw = np.asarray(q_weight, dtype=np.float32)
    kw = np.asarray(k_weight, dtype=np.float32)
    vw = np.asarray(v_weight, dtype=np.float32)
    ow = np.asarray(o_weight, dtype=np.float32)

    import ml_dtypes
    xT = np.ascontiguousarray(x.reshape(BS, D_MODEL).T).astype(ml_dtypes.bfloat16)

    perm64 = np.concatenate([np.arange(0, 64, 2), np.arange(1, 64, 2)])

    half = D_HEAD // 2
    inv_freq = THETA ** (-(np.arange(half, dtype=np.float64) * 2.0 / D_HEAD))
    pos = np.arange(S, dtype=np.float64)
    ang = pos[None, :] * inv_freq[:, None]        # [32, S]
    angf = np.tile(ang, (4, 1))                   # [128, S], row p -> i = p % 32
    trig = np.ascontiguousarray(np.stack(
        [np.cos(angf), np.sin(angf)], axis=1).astype(np.float32))

    spermT = np.zeros((128, 128), dtype=np.float32)
    for h in range(2):
        for i in range(32):
            spermT[h * 64 + 32 + i, h * 64 + i] = -1.0
            spermT[h * 64 + i, h * 64 + 32 + i] = 1.0

    kq = np.arange(128)
    mask128 = (np.arange(128)[None, :] >= kq[:, None]).astype(ml_dtypes.bfloat16)

    shared = dict(xT=xT, trig=trig, sperm=spermT, mask128=mask128)

    in_maps = []
    for c in range(N_CORES):
        rows = slice(128 * c, 128 * (c + 1))

        def permqk(w):
            wc = w[rows]
            return np.ascontiguousarray(
                np.concatenate([wc[0:64][perm64], wc[64:128][perm64]]).T
            ).astype(ml_dtypes.bfloat16)

        in_maps.append(dict(
            shared,
            wq=permqk(qw),
            wk=permqk(kw),
            wv=np.ascontiguousarray(vw[rows].T).astype(ml_dtypes.bfloat16),
            wo=np.ascontiguousarray(ow[:, rows].T),
        ))
    return in_maps


def kernel(q_weight, k_weight, v_weight, o_weight, in_features):
    rt = _build()
    in_maps = _host_inputs(q_weight, k_weight, v_weight, o_weight, in_features)
    res = rt["run_bass_kernel_spmd"](
        rt["nc"], in_maps, core_ids=list(range(N_CORES)),
    )
    y = np.zeros((BS, D_MODEL), dtype=np.float32)
    for c in range(N_CORES):
        y += np.asarray(res.results[c]["y"], dtype=np.float32)
    return y.reshape(B, S, D_MODEL)


# revision 16
# speedup vs baseline: 1.0287x; 1.0022x over previous
"""Trainium2 Bass kernel for multi-head attention (B=2, S=2048, D=1024, H=16, causal, RoPE).

Sharding: tensor-parallel over heads. Each of the 8 cores computes 2 heads
(128 of the 1024 q/k/v dims): QKV projections for its head slice, RoPE,
causal attention, and a partial output projection against its 128-column
slice of o_weight. The host sums the 8 partial outputs (the all-reduce).

Device-side layout choices:
  - Activations live transposed: q/k are [128 (head dims), seq] so the
    scores matmul contracts dh on partitions. RoPE pairs are de-interleaved
    on the host (weight-row permutation) so pair partners sit 32 partitions
    apart; the rotate step is a single 128x128 sign-swap matmul (sperm).
  - Scores run as fp8e4 DoubleRow matmuls at half cost: stationary is
    [64, 2, 128] = (fp8(k), k - fp8(k)) so the k-side quantization error
    cancels; the moving q8 is a stride-0 broadcast over the pair dim.
  - Scores are computed transposed ([sk, sq]) so P = exp(scores) feeds the
    PV matmul directly as the moving operand (bf16); the two heads' score
    matmuls run per-head. V carries a block of 64 ones columns, so the PV
    matmul also emits the softmax denominator replicated across 64
    partitions; normalization is then a plain reciprocal+multiply.
  - V is projected directly into [seq, dh] layout by using the (transposed)
    x tiles as the stationary operand - no on-chip transposes needed.
  - Work is software-pipelined: projection chunks run one chunk ahead of
    attention, output projections trail one chunk behind, both threaded
    through the attention tile loop so TensorE, ScalarE (exp), VectorE and
    GpSimd stay concurrently busy.
  - Partial outputs are written in bf16 (summed in fp32 on the host).
"""

import numpy as np

D_MODEL = 1024
N_HEADS = 16
D_HEAD = 64
THETA = 10000.0
B = 2
S = 2048
N_CORES = 8
BS = B * S  # 4096
NQ = 512    # query chunk width
NK = 128    # key tile width

_RT = {}


def _build():
    if _RT:
        return _RT
    import sys
    try:
        import concourse.bass  # noqa: F401
    except ImportError:
        sys.path.insert(0, "/opt/trn_rl_repo")
    import concourse.mybir as mybir
    import concourse.tile as tile
    from concourse import bacc
    from concourse._compat import axon_active
    from concourse.bass_utils import run_bass_kernel_spmd

    f32 = mybir.dt.float32
    f32r = mybir.dt.float32r
    bf16 = mybir.dt.bfloat16
    fp8 = mybir.dt.float8e4
    EXP = mybir.ActivationFunctionType.Exp
    DR = mybir.MatmulPerfMode.DoubleRow

    nc = bacc.Bacc(
        "TRN2", target_bir_lowering=False, debug=not axon_active(),
        num_devices=N_CORES,
    )

    xT = nc.dram_tensor("xT", [D_MODEL, BS], bf16, kind="ExternalInput").ap()
    wq = nc.dram_tensor("wq", [D_MODEL, 128], bf16, kind="ExternalInput").ap()
    wk = nc.dram_tensor("wk", [D_MODEL, 128], bf16, kind="ExternalInput").ap()
    wv = nc.dram_tensor("wv", [D_MODEL, 128], bf16, kind="ExternalInput").ap()
    wo = nc.dram_tensor("wo", [128, D_MODEL], f32r, kind="ExternalInput").ap()
    trig = nc.dram_tensor("trig", [128, 2, S], f32, kind="ExternalInput").ap()
    sperm = nc.dram_tensor("sperm", [128, 128], f32r, kind="ExternalInput").ap()
    mask128 = nc.dram_tensor("mask128", [128, 128], bf16, kind="ExternalInput").ap()
    y = nc.dram_tensor("y", [BS, D_MODEL], bf16, kind="ExternalOutput").ap()

    with tile.TileContext(nc) as tc:
        with (
            tc.tile_pool(name="singles", bufs=1) as singles,
            tc.tile_pool(name="px", bufs=3) as px,
            tc.tile_pool(name="ptmp", bufs=3) as ptmp,
            tc.tile_pool(name="pp", bufs=4) as pp,
            tc.tile_pool(name="pys", bufs=4) as pys,
            tc.tile_pool(name="pr", bufs=3) as pr,
            tc.tile_pool(name="ps_a", bufs=2, space="PSUM") as ps_a,
            tc.tile_pool(name="ps_s", bufs=2, space="PSUM") as ps_s,
            tc.tile_pool(name="ps_o", bufs=2, space="PSUM") as ps_o,
        ):
            wq_sb = singles.tile([128, 8, 128], bf16, tag="wq")
            wk_sb = singles.tile([128, 8, 128], bf16, tag="wk")
            wv_sb = singles.tile([128, 8, 128], bf16, tag="wv")
            wo_sb = singles.tile([128, D_MODEL], f32r, tag="wo")
            sperm_sb = singles.tile([128, 128], f32r, tag="sperm")
            m128_sb = singles.tile([128, 128], bf16, tag="m128")
            warm_sb = singles.tile([1, 2], f32, tag="warm")
            # q8: rope'd q in fp8, one slot per chunk-within-batch.
            q8_sb = singles.tile([128, 4, NQ], fp8, tag="q8")
            # k8: [head dims, batch parity, (fp8(k), k-fp8(k)), key pos]
            k8_sb = singles.tile([128, 2, 2, S], fp8, tag="k8")
            oT_sb = singles.tile([128, BS], f32r, tag="oT")
            # V tiles: [seq-tile partitions, 32 tiles, 192]: cols 0:64 head A,
            # 64:128 ones, 128:192 head B. Head A lhsT = cols 0:128, head B
            # lhsT = cols 64:192; the ones block replicates the denominator.
            v_sb = singles.tile([128, 32, 192], bf16, tag="v")

            junk_sb = singles.tile([128, 512], f32r, tag="junk")

            nc.scalar.dma_start(out=wq_sb, in_=wq.rearrange("(a p) m -> p a m", p=128))
            nc.scalar.dma_start(out=wk_sb, in_=wk.rearrange("(a p) m -> p a m", p=128))
            nc.scalar.dma_start(out=sperm_sb, in_=sperm)
            nc.scalar.dma_start(out=wv_sb, in_=wv.rearrange("(a p) m -> p a m", p=128))
            nc.scalar.dma_start(out=m128_sb, in_=mask128)
            nc.vector.memset(v_sb[:, :, 64:128].bitcast(bf16), 1.0)

            # warm the ACT exp table before the first real exp
            nc.vector.memset(warm_sb, 0.0)
            nc.scalar.activation(warm_sb[:, 0:1], warm_sb[:, 1:2], EXP)

            # dummy matmuls: keep PE busy during the initial DMAs and ramp
            # the p-state clock before the first real matmul arrives
            nc.gpsimd.memset(junk_sb, 0.0)
            for w in range(6):
                jp = ps_s.tile([128, 2, 512], f32, tag="sps")
                nc.tensor.matmul(jp[:, 0, :], junk_sb[:, 0:128], junk_sb,
                                 start=True, stop=True)

            def late_consts():
                nc.sync.dma_start(out=wo_sb, in_=wo)

            def proj_pieces(b, c, after_xt=None):
                """QKV projections + rope + V transpose for seq chunk c of
                batch b (512 positions), as a list of closures that can be
                threaded through the attention tile loop."""
                u = 4 * b + c
                s0 = 512 * c
                xt = []
                tmps = {}
                tg = [None]

                def p_load():
                    for h in range(2):
                        xth = px.tile([128, 4, 512], bf16, tag=f"xt{h}")
                        nc.sync.dma_start(
                            out=xth,
                            in_=xT[512 * h:512 * (h + 1), 512 * u:512 * (u + 1)]
                            .rearrange("(a p) n -> p a n", p=128),
                        )
                        xt.append(xth)
                    tg[0] = ptmp.tile([128, 2, 512], f32, tag="tg", name="tg")
                    nc.sync.dma_start(out=tg[0], in_=trig[:, :, s0:s0 + 512])
                    if after_xt is not None:
                        after_xt()

                def xslot(d):
                    return xt[d // 4][:, d % 4, :]

                def p_proj(w_sb, name):
                    def f():
                        ps = ps_a.tile([128, 512], f32, tag="pa")
                        for d in range(8):
                            nc.tensor.matmul(
                                ps, w_sb[:, d, :], xslot(d),
                                start=(d == 0), stop=(d == 7),
                            )
                        tmp = ptmp.tile([128, 512], f32r, tag=f"{name}tmp")
                        nc.gpsimd.tensor_copy(tmp, ps)  # PSUM -> SBUF
                        tmps[name] = tmp
                    return f

                def p_rope_q():
                    tmp = tmps["q"]
                    sq = ps_a.tile([128, 512], f32, tag="pa")
                    nc.tensor.matmul(sq, sperm_sb, tmp, start=True, stop=True)
                    cs = tg[0][:, 0, :]
                    sn = tg[0][:, 1, :]
                    m1 = ptmp.tile([128, 512], f32, tag="m1")
                    m2 = ptmp.tile([128, 512], f32, tag="m2")
                    nc.gpsimd.tensor_mul(m1, tmp, cs)
                    nc.vector.tensor_mul(m2, sq, sn)
                    nc.gpsimd.tensor_add(q8_sb[:, c, :], m1, m2)

                def p_rope_k():
                    tmp = tmps["k"]
                    sq = ps_a.tile([128, 512], f32, tag="pa")
                    nc.tensor.matmul(sq, sperm_sb, tmp, start=True, stop=True)
                    cs = tg[0][:, 0, :]
                    sn = tg[0][:, 1, :]
                    m1 = ptmp.tile([128, 512], f32, tag="m1")
                    kf = ptmp.tile([128, 512], f32, tag="kf")
                    nc.gpsimd.tensor_mul(m1, tmp, cs)
                    nc.vector.tensor_mul(kf, sq, sn)
                    nc.gpsimd.tensor_add(kf, m1, kf)
                    k8h = k8_sb[:, b % 2, 0, s0:s0 + 512]
                    nc.vector.tensor_copy(k8h, kf)
                    nc.gpsimd.tensor_sub(k8_sb[:, b % 2, 1, s0:s0 + 512], kf, k8h)

                vps = [None]

                def p_vproj(subs):
                    def f():
                        if vps[0] is None:
                            vps[0] = ps_a.tile([128, 4, 128], f32, tag="pa",
                                               name="vps")
                        for sub in subs:
                            for d in range(8):
                                nc.tensor.matmul(
                                    vps[0][:, sub, :],
                                    xslot(d)[:, 128 * sub:128 * (sub + 1)],
                                    wv_sb[:, d, :],
                                    start=(d == 0), stop=(d == 7),
                                )
                    return f

                def p_vstore_a():
                    nc.vector.tensor_copy(v_sb[:, 4 * u:4 * u + 4, 0:64],
                                          vps[0][:, :, 0:64])

                def p_vstore_b():
                    nc.vector.tensor_copy(v_sb[:, 4 * u:4 * u + 4, 128:192],
                                          vps[0][:, :, 64:128])

                return [p_load, p_proj(wq_sb, "q"), p_proj(wk_sb, "k"),
                        p_rope_q, p_vproj((0,)), p_vproj((1,)),
                        p_rope_k, p_vproj((2,)), p_vproj((3,)),
                        p_vstore_a, p_vstore_b]

            def proj_chunk(b, c, after_xt=None):
                for f in proj_pieces(b, c, after_xt):
                    f()

            def oproj_piece(b, c, s4, late=False):
                """Output projection for one 128-row seq tile (emitted one
                chunk late, spread across the next chunk's tiles)."""
                row0 = S * b + NQ * c + 128 * s4
                yp = ps_s.tile([128, 2, 512], f32, tag="sps")
                for hn in range(2):
                    nc.tensor.matmul(
                        yp[:, hn, :],
                        oT_sb[:, row0:row0 + 128],
                        wo_sb[:, 512 * hn:512 * (hn + 1)],
                        start=True, stop=True,
                    )
                ys = pys.tile([128, 1024], bf16, tag="ys")
                # split halves across two engines so the PSUM slot frees
                # fast; keep ACT free of copies in the exp-bound late chunks
                nc.vector.tensor_copy(ys[:, 0:512], yp[:, 0, :])
                if late:
                    nc.gpsimd.tensor_copy(ys[:, 512:1024], yp[:, 1, :])
                else:
                    nc.scalar.copy(ys[:, 512:1024], yp[:, 1, :])
                nc.sync.dma_start(out=y[row0:row0 + 128, :], in_=ys)

            def attn_chunk(b, c, mids=(), fine_tail=False, group_tail=False):
                """Causal attention for query chunk c of batch b. ``mids`` are
                emitted one per attention tile (pipelined filler work such as
                the previous chunk's output projection). With ``group_tail``
                (final chunk), PV accumulation stops per 128-query column
                group so normalize+oproj+store overlap the remaining tiles."""
                mids = list(mids)
                qsl = slice(S * b + NQ * c, S * b + NQ * (c + 1))
                nt = (NQ // NK) * (c + 1)
                oa = ps_o.tile([128, 512], f32, tag="oacc")
                ob = ps_o.tile([128, 512], f32, tag="oacc")
                pending = []  # (p tile, j, t) awaiting PV matmul
                PV_DEPTH = 2

                def finish_group(g):
                    # group g of this chunk is fully accumulated: normalize,
                    # project, store - all while later tiles still run
                    fs = slice(128 * g, 128 * (g + 1))
                    qs4 = slice(qsl.start + 128 * g, qsl.start + 128 * (g + 1))
                    rra = pr.tile([64, 128], f32, tag="rra")
                    rrb = pr.tile([64, 128], f32, tag="rrb")
                    nc.vector.reciprocal(rra, oa[64:128, fs])
                    nc.vector.tensor_mul(oT_sb[0:64, qs4], oa[0:64, fs], rra)
                    nc.vector.reciprocal(rrb, ob[0:64, fs])
                    nc.vector.tensor_mul(oT_sb[64:128, qs4], ob[64:128, fs], rrb)
                    oproj_piece(b, c, g, late=True)

                def pv_flush():
                    p, j, _t = pending.pop(0)
                    w0 = 128 * j
                    if group_tail:
                        for g in range(j, 4):
                            gs = slice(128 * g, 128 * (g + 1))
                            nc.tensor.matmul(
                                oa[:, gs], v_sb[:, 16 * b + _t, 0:128],
                                p[:, 0, gs],
                                start=(_t == 0), stop=(_t == 4 * c + g),
                            )
                            nc.tensor.matmul(
                                ob[:, gs], v_sb[:, 16 * b + _t, 64:192],
                                p[:, 1, gs],
                                start=(_t == 0), stop=(_t == 4 * c + g),
                            )
                        if _t >= 4 * c:
                            finish_group(_t - 4 * c)
                        return
                    nc.tensor.matmul(
                        oa[:, w0:512], v_sb[:, 16 * b + _t, 0:128],
                        p[:, 0, w0:512],
                        start=(_t == 0), stop=(_t == nt - 1),
                    )
                    nc.tensor.matmul(
                        ob[:, w0:512], v_sb[:, 16 * b + _t, 64:192],
                        p[:, 1, w0:512],
                        start=(_t == 0), stop=(_t == nt - 1),
                    )

                for t in range(nt):
                    j = max(0, t - 4 * c)  # within-chunk diagonal offset
                    w0 = 128 * j           # causally-dead query columns
                    sps = ps_s.tile([128, 2, 512], f32, tag="sps")
                    for h in range(2):
                        hs = slice(64 * h, 64 * h + 64)
                        nc.tensor.matmul(
                            sps[:, h, w0:512],
                            k8_sb[hs, b % 2, :, 128 * t:128 * (t + 1)],
                            q8_sb[hs, c, w0:512].unsqueeze(1)
                            .to_broadcast([64, 2, 512 - w0]),
                            start=True, stop=True, perf_mode=DR,
                        )
                    p = pp.tile([128, 2, 512], bf16, tag="p")
                    nc.scalar.activation(
                        p[:, :, w0:512], sps[:, :, w0:512], EXP, scale=0.125,
                    )
                    if t >= 4 * c:  # diagonal tile: mask boundary block
                        pb = p[:, :, w0:w0 + 128]
                        nc.vector.tensor_mul(
                            pb, pb, m128_sb.unsqueeze(1).to_broadcast([128, 2, 128]),
                        )
                    if len(pending) >= PV_DEPTH:
                        pv_flush()
                    pending.append((p, j, t))
                    if mids:
                        mids.pop(0)()
                while pending:
                    pv_flush()
                for m in mids:  # in case nt < len(mids)
                    m()
                if group_tail:
                    return

                # oa rows 64:128 / ob rows 0:64 hold the replicated
                # softmax denominators (from the ones block in V).
                rra = pr.tile([64, 512], f32, tag="rra")
                rrb = pr.tile([64, 512], f32, tag="rrb")
                if fine_tail:
                    # per-seq-tile normalize so the trailing output projection
                    # can start before the whole chunk is normalized
                    for s4 in range(4):
                        fs = slice(128 * s4, 128 * (s4 + 1))
                        qs4 = slice(qsl.start + 128 * s4, qsl.start + 128 * (s4 + 1))
                        nc.vector.reciprocal(rra[:, fs], oa[64:128, fs])
                        nc.vector.tensor_mul(oT_sb[0:64, qs4], oa[0:64, fs],
                                             rra[:, fs])
                        nc.vector.reciprocal(rrb[:, fs], ob[0:64, fs])
                        nc.vector.tensor_mul(oT_sb[64:128, qs4], ob[64:128, fs],
                                             rrb[:, fs])
                else:
                    nc.vector.reciprocal(rra, oa[64:128, :])
                    nc.vector.reciprocal(rrb, ob[0:64, :])
                    nc.vector.tensor_mul(oT_sb[0:64, qsl], oa[0:64, :], rra)
                    nc.vector.tensor_mul(oT_sb[64:128, qsl], ob[64:128, :], rrb)

            # Software pipeline: projections run one chunk ahead of attention;
            # output projections trail their attention chunk by one.
            def oproj_mids(bc, late=False):
                if bc is None:
                    return ()
                return [lambda s4=s4: oproj_piece(bc[0], bc[1], s4, late=late)
                        for s4 in range(4)]

            prev = None  # (b, c) whose oproj is still owed
            for b in range(B):
                if b == 0:
                    proj_chunk(b, 0, after_xt=late_consts)
                for c in range(4):
                    mids = list(oproj_mids(prev, late=(b == 1 and c >= 2)))
                    if c + 1 < 4:
                        pieces = proj_pieces(b, c + 1)
                    elif b + 1 < B:
                        # thread the next batch's first projection through
                        # this batch's last attention chunk
                        pieces = proj_pieces(b + 1, 0)
                    else:
                        pieces = []
                    merged = []
                    while pieces or mids:
                        if pieces:
                            merged.append(pieces.pop(0))
                        if mids:
                            merged.append(mids.pop(0))
                    mids = merged
                    last = b + 1 == B and c == 3
                    attn_chunk(b, c, mids=mids, group_tail=last)
                    prev = (b, c)

    nc.compile()
    _RT.update(
        nc=nc, run_bass_kernel_spmd=run_bass_kernel_spmd, mybir=mybir,
    )
    return _RT


def _host_inputs(q_weight, k_weight, v_weight, o_weight, in_features):
    """Build the per-core input maps (host-side sharding + layout prep)."""
    x = np.ascontiguousarray(np.asarray(in_features, dtype=np.float32))
    qw = np.asarray(q_weight, dtype=np.float32)
    kw = np.asarray(k_weight, dtype=np.float32)
    vw = np.asarray(v_weight, dtype=np.float32)
    ow = np.asarray(o_weight, dtype=np.float32)

    import ml_dtypes
    xT = np.ascontiguousarray(x.reshape(BS, D_MODEL).T).astype(ml_dtypes.bfloat16)

    perm64 = np.concatenate([np.arange(0, 64, 2), np.arange(1, 64, 2)])

    half = D_HEAD // 2
    inv_freq = THETA ** (-(np.arange(half, dtype=np.float64) * 2.0 / D_HEAD))
    pos = np.arange(S, dtype=np.float64)
    ang = pos[None, :] * inv_freq[:, None]        # [32, S]
    angf = np.tile(ang, (4, 1))                   # [128, S], row p -> i = p % 32
    trig = np.ascontiguousarray(np.stack(
        [np.cos(angf), np.sin(angf)], axis=1).astype(np.float32))

    spermT = np.zeros((128, 128), dtype=np.float32)
    for h in range(2):
        for i in range(32):
            spermT[h * 64 + 32 + i, h * 64 + i] = -1.0
            spermT[h * 64 + i, h * 64 + 32 + i] = 1.0

    kq = np.arange(128)
    mask128 = (np.arange(128)[None, :] >= kq[:, None]).astype(ml_dtypes.bfloat16)

    shared = dict(xT=xT, trig=trig, sperm=spermT, mask128=mask128)

    in_maps = []
    for c in range(N_CORES):
        rows = slice(128 * c, 128 * (c + 1))

        def permqk(w):
            wc = w[rows]
            return np.ascontiguousarray(
                np.concatenate([wc[0:64][perm64], wc[64:128][perm64]]).T
            ).astype(ml_dtypes.bfloat16)

        in_maps.append(dict(
            shared,
            wq=permqk(qw),
            wk=permqk(kw),
            wv=np.ascontiguousarray(vw[rows].T).astype(ml_dtypes.bfloat16),
            wo=np.ascontiguousarray(ow[:, rows].T),
        ))
    return in_maps


def kernel(q_weight, k_weight, v_weight, o_weight, in_features):
    rt = _build()
    in_maps = _host_inputs(q_weight, k_weight, v_weight, o_weight, in_features)
    res = rt["run_bass_kernel_spmd"](
        rt["nc"], in_maps, core_ids=list(range(N_CORES)),
    )
    y = np.zeros((BS, D_MODEL), dtype=np.float32)
    for c in range(N_CORES):
        y += np.asarray(res.results[c]["y"], dtype=np.float32)
    return y.reshape(B, S, D_MODEL)
